# revision 5
# baseline (speedup 1.0000x reference)
"""Trainium2 Bass kernel for DirectionalSeparableConv2D.

Full-input contract: kernel(**inputs) takes the complete unsharded inputs
(x [128,128,48,48] plus the small weight tensors) and returns the full
[128,128,48,48] output. Internally shards batch 16-per-core across 8
NeuronCores (SPMD), with all weights replicated.

Math: the whole module is linear in x.
  stage 1 (depthwise): central group (ch 0:32) 3x3 kernels; four
    directional groups (24 ch each) 5-tap line kernels along
    horizontal / vertical / diagonal / anti-diagonal directions.
  stage 2: 1x1 mixing, representable as a single [128,128] matrix Mix.

Kernel strategy per core:
  - channels on SBUF partitions, pixels on the free dim.
  - 4-image blocks: image i occupies partition slot [32i, 32i+32).
  - TensorE: for tap s handled on PE, out += (Mix*diag(k_s)) @ shift_s(x)
    as PSUM-accumulated matmuls; shift_s is a free-dim AP offset with
    border-clipped ranges (SAME padding falls out). 4 images run
    concurrently via 32-row tile_position packing. PE covers all 9
    central-group taps, the center (t=2) and outermost (t=4) tap of
    each directional group, and the stage-2 mixing of the directional
    depthwise partials.
  - VectorE/ScalarE: directional taps t=0 (ScalarE mult head) and
    t=1,3 (VectorE fused scalar_tensor_tensor MACs) accumulate y' in
    SBUF; ScalarE also evacuates PSUM->SBUF. GpSimd zeroes the tiny
    border strips the t=0 head cannot reach.
  - DMA: ~37.7MB per core => ~105us at ~358GB/s; everything else is
    sized to hide underneath that.
"""

import numpy as np

import concourse.bacc as bacc
import concourse.mybir as mybir
import concourse.tile as tile
from concourse.bass_utils import run_bass_kernel_spmd

F32 = mybir.dt.float32

# Problem shapes (hardcoded per contract).
B, C, H, W = 128, 128, 48, 48
HW = H * W
CEN_IN, DIR_IN = 32, 24
N_CORES = 8

# Per-core tiling.
NB = B // N_CORES          # images per core (16)
IPB = 4                    # images per block (one per 32-partition slot)
RPC = 8                    # rows per PSUM chunk
CHUNK = RPC * W            # 384 <= 512 fp32 PSUM bank
NCH = H // RPC             # 6 chunks per image
PR = 96 + DIR_IN           # partition extent covering all 4 slots' real dir channels

# Dir-group tap geometry: group g shift for tap t (t=0..4, center t=2).
#   g=0 horizontal (0, t-2); g=1 vertical (t-2, 0);
#   g=2 diagonal (t-2, t-2); g=3 anti-diagonal (t-2, 2-t).
def dir_shift(g, t):
    d = t - 2
    return [(0, d), (d, 0), (d, d), (d, -d)][g]

DVE_TAPS = [0, 1, 3]       # dir taps on ACT(head)/DVE; t=2,4 go to PE

# Weight bundle free-dim layout: 21 PE lhsT blocks of 128 + scalar cols.
W_CEN = 0                  # 9 blocks: central tap t=0..8, (dy,dx)=(t//3-1, t%3-1)
W_A0 = 9 * 128             # 4 blocks: dir-group center taps (t=2)
W_S2 = 13 * 128            # 4 blocks: dir-group stage-2 mixing
W_D4 = 17 * 128            # 4 blocks: dir-group outermost taps (t=4)
W_SCAL = 21 * 128          # scalar cols for DVE_TAPS
NWT = W_SCAL + len(DVE_TAPS)

XPAD = 8 * HW              # trailing DRAM pad so 32-wide dir-slot loads stay in bounds


def build_mix(cen2cen, par2cen, dia2cen, cen2dir, dir2dir):
    mix = np.zeros((C, C), np.float32)
    mix[0:32, 0:32] = cen2cen
    mix[0:32, 32:56] = par2cen
    mix[0:32, 56:80] = par2cen
    mix[0:32, 80:104] = dia2cen
    mix[0:32, 104:128] = dia2cen
    for g in range(4):
        r = 32 + 24 * g
        mix[r:r + 24, 0:32] = cen2dir
        mix[r:r + 24, r:r + 24] = dir2dir
    return mix


def build_weights(cen_tensor, dir_tensor, cen2cen, par2cen, dia2cen, cen2dir, dir2dir):
    mix = build_mix(cen2cen, par2cen, dia2cen, cen2dir, dir2dir)
    wt = np.zeros((128, NWT), np.float32)
    for t in range(9):
        # lhsT[k, m] = Mix[m, k] * cen_k[t]
        blk = (mix[:, 0:32] * cen_tensor[:, t // 3, t % 3][None, :]).T
        for i in range(IPB):
            wt[32 * i:32 * i + 32, W_CEN + 128 * t:W_CEN + 128 * (t + 1)] = blk
    for g in range(4):
        cols = slice(32 + 24 * g, 56 + 24 * g)
        a0 = (mix[:, cols] * dir_tensor[:, 2][None, :]).T
        s2 = mix[:, cols].T
        d4 = (mix[:, cols] * dir_tensor[:, 4][None, :]).T
        for i in range(IPB):
            wt[32 * i:32 * i + 24, W_A0 + 128 * g:W_A0 + 128 * (g + 1)] = a0
            wt[32 * i:32 * i + 24, W_S2 + 128 * g:W_S2 + 128 * (g + 1)] = s2
            wt[32 * i:32 * i + 24, W_D4 + 128 * g:W_D4 + 128 * (g + 1)] = d4
    for j, t in enumerate(DVE_TAPS):
        for i in range(IPB):
            wt[32 * i:32 * i + 24, W_SCAL + j] = dir_tensor[:, t]
    return wt


def build_nc(nb=NB):
    """Emit the per-core Bass program for nb images."""
    assert nb % IPB == 0
    nblk = nb // IPB
    nc = bacc.Bacc("TRN2", target_bir_lowering=False, debug=False)

    x = nc.dram_tensor("x", [nb * C * HW + XPAD], F32, kind="ExternalInput")
    wtd = nc.dram_tensor("wt", [128, NWT], F32, kind="ExternalInput")
    out = nc.dram_tensor("out", [nb, C, HW], F32, kind="ExternalOutput")

    # Flat channel view: row bc = image (bc//128), channel (bc%128).
    xc = x[:].rearrange("(bc f) -> bc f", f=HW)

    with tile.TileContext(nc) as tc:
        with (
            tc.tile_pool(name="wpool", bufs=1) as wpool,
            tc.tile_pool(name="xpool", bufs=2) as xpool,
            tc.tile_pool(name="ypool", bufs=2) as ypool,
            tc.tile_pool(name="spool", bufs=3) as spool,
            tc.tile_pool(name="ppool", bufs=8, space="PSUM") as ppool,
        ):
            wtile = wpool.tile([128, NWT], F32)
            nc.sync.dma_start(out=wtile[:, :], in_=wtd[:, :])
            scal = [wtile[0:PR, W_SCAL + j:W_SCAL + j + 1] for j in range(len(DVE_TAPS))]

            for blk in range(nblk):
                b0 = blk * IPB
                cen4 = xpool.tile([128, HW], F32, name=f"cen4_{blk}", tag="cen4")
                dir4 = xpool.tile([128, 4 * HW], F32, name=f"dir4_{blk}", tag="dir4")
                y4 = ypool.tile([128, 4 * HW], F32, name=f"y4_{blk}", tag="y4")

                # ---- loads: channels -> partition slots -------------------
                for i in range(IPB):
                    bc = (b0 + i) * C
                    nc.sync.dma_start(
                        out=cen4[32 * i:32 * i + 32, :],
                        in_=xc[bc:bc + 32, :],
                    )
                    for g in range(4):
                        # 32-wide load: 24 real channels + 8 don't-care.
                        c0 = bc + 32 + 24 * g
                        nc.sync.dma_start(
                            out=dir4[32 * i:32 * i + 32, g * HW:(g + 1) * HW],
                            in_=xc[c0:c0 + 32, :],
                        )

                # ---- stage 1 dir taps t=0,1,3 on ACT/DVE ------------------
                d4v = dir4[:, :].rearrange("p (g h w) -> p g h w", g=4, w=W)
                y4v = y4[:, :].rearrange("p (g h w) -> p g h w", g=4, w=W)

                # Head tap (t=0) writes its valid region; memset the
                # complement strips so y' is fully initialized.
                #   g=0 (0,-2): cols 0:2
                #   g=1 (-2,0): rows 0:2
                #   g=2 (-2,-2): rows 0:2 + cols 0:2 (rows 2:)
                #   g=3 (-2, 2): rows 0:2 + cols 46:48 (rows 2:)
                nc.gpsimd.memset(y4v[0:PR, 0, :, 0:2], 0.0)
                nc.gpsimd.memset(y4v[0:PR, 1, 0:2, :], 0.0)
                nc.gpsimd.memset(y4v[0:PR, 2, 0:2, :], 0.0)
                nc.gpsimd.memset(y4v[0:PR, 2, 2:H, 0:2], 0.0)
                nc.gpsimd.memset(y4v[0:PR, 3, 0:2, :], 0.0)
                nc.gpsimd.memset(y4v[0:PR, 3, 2:H, W - 2:W], 0.0)

                for g in range(4):
                    for j, t in enumerate(DVE_TAPS):
                        dy, dx = dir_shift(g, t)
                        rl, rh = max(0, -dy), H - max(0, dy)
                        cl, ch = max(0, -dx), W - max(0, dx)
                        src = d4v[0:PR, g, rl + dy:rh + dy, cl + dx:ch + dx]
                        dst = y4v[0:PR, g, rl:rh, cl:ch]
                        if t == 0:
                            # chain head: y = k * x_s  (ScalarE)
                            nc.scalar.mul(dst, src, scal[j])
                        else:
                            nc.vector.scalar_tensor_tensor(
                                out=dst, in0=src, scalar=scal[j], in1=dst,
                                op0=mybir.AluOpType.mult, op1=mybir.AluOpType.add,
                            )

                # ---- PE taps + mixing, chunked over pixels ----------------
                cen4v = cen4[:, :].rearrange("p (h w) -> p h w", w=W)
                for chk in range(NCH):
                    r0 = chk * RPC
                    c0 = r0 * W
                    pt = [
                        ppool.tile([128, CHUNK], F32, name=f"ps_{blk}_{chk}_{i}", tag="ps")
                        for i in range(IPB)
                    ]
                    ptv = [p[:, :].rearrange("p (h w) -> p h w", w=W) for p in pt]

                    def mm_tap(wcol, kk, rhs_tile, rhs_v, goff, dy, dx, first=False):
                        """One shifted tap for all IPB images (row-packed).

                        rhs free-dim AP is clipped to the SAME-padding-valid
                        region; the PSUM out AP targets the matching region.
                        Contiguous (full-width) regions use flat 1D APs.
                        """
                        rl = max(r0, -dy)
                        rh = min(r0 + RPC, H - max(0, dy))
                        cl, ch = max(0, -dx), W - max(0, dx)
                        wsl = wtile[:, wcol:wcol + 128]
                        for i in range(IPB):
                            p0 = 32 * i
                            if cl == 0 and ch == W:
                                o = pt[i][:, rl * W - c0:rh * W - c0]
                                r = rhs_tile[p0:p0 + kk,
                                             goff + (rl + dy) * W:goff + (rh + dy) * W]
                            else:
                                o = ptv[i][:, rl - r0:rh - r0, cl:ch]
                                r = rhs_v[p0:p0 + kk, rl + dy:rh + dy, cl + dx:ch + dx]
                            nc.tensor.matmul(
                                o, wsl[p0:p0 + kk, :], r,
                                start=first, stop=False, tile_position=(p0, 0),
                            )

                    # central center tap first: full coverage, opens the group
                    mm_tap(W_CEN + 128 * 4, 32, cen4, cen4v, 0, 0, 0, first=True)
                    for t in range(9):
                        if t == 4:
                            continue
                        mm_tap(W_CEN + 128 * t, 32, cen4, cen4v,
                               0, t // 3 - 1, t % 3 - 1)
                    # dir-group center (t=2) and outermost (t=4) taps
                    for g in range(4):
                        dgv = d4v[:, g]
                        mm_tap(W_A0 + 128 * g, 24, dir4, dgv, g * HW, 0, 0)
                    for g in range(4):
                        dgv = d4v[:, g]
                        dy, dx = dir_shift(g, 4)
                        mm_tap(W_D4 + 128 * g, 24, dir4, dgv, g * HW, dy, dx)
                    # stage-2 mixing of dir depthwise partials (y': t=0,1,3)
                    for g in range(4):
                        for i in range(IPB):
                            nc.tensor.matmul(
                                pt[i][:, :],
                                wtile[32 * i:32 * i + 24, W_S2 + 128 * g:W_S2 + 128 * (g + 1)],
                                y4[32 * i:32 * i + 24, g * HW + c0:g * HW + c0 + CHUNK],
                                start=False, stop=(g == 3), tile_position=(32 * i, 0),
                            )

                    # evacuate + store (4 images coalesced)
                    stag = spool.tile([128, IPB * CHUNK], F32, name=f"st_{blk}_{chk}", tag="st")
                    for i in range(IPB):
                        nc.scalar.copy(stag[:, i * CHUNK:(i + 1) * CHUNK], pt[i][:, :])
                    dst = out[b0:b0 + IPB, :, c0:c0 + CHUNK].transpose([1, 0, 2])
                    src = stag[:, :].rearrange("p (i f) -> p i f", i=IPB)
                    nc.sync.dma_start(out=dst, in_=src)

    nc.compile()
    return nc


_NC_CACHE = {}


def _get_nc(nb):
    if nb not in _NC_CACHE:
        _NC_CACHE[nb] = build_nc(nb)
    return _NC_CACHE[nb]


def make_in_maps(x, wt_np, nb=NB, n_cores=N_CORES):
    x = np.ascontiguousarray(x, np.float32).reshape(B, C, HW)
    in_maps = []
    for k in range(n_cores):
        xs = x[k * nb:(k + 1) * nb].ravel()
        xs = np.concatenate([xs, np.zeros(XPAD, np.float32)])
        in_maps.append({"x": xs, "wt": wt_np})
    return in_maps


def kernel(x, cen_tensor, dir_tensor, cen2cen, par2cen, dia2cen, cen2dir, dir2dir,
           _trace=False):
    wt_np = build_weights(
        np.asarray(cen_tensor, np.float32), np.asarray(dir_tensor, np.float32),
        np.asarray(cen2cen, np.float32), np.asarray(par2cen, np.float32),
        np.asarray(dia2cen, np.float32), np.asarray(cen2dir, np.float32),
        np.asarray(dir2dir, np.float32))
    nc = _get_nc(NB)
    in_maps = make_in_maps(np.asarray(x), wt_np)
    res = run_bass_kernel_spmd(nc, in_maps, list(range(N_CORES)), trace=_trace)
    outs = [res.results[k]["out"].reshape(NB, C, H, W) for k in range(N_CORES)]
    full = np.concatenate(outs, axis=0)
    if _trace:
        return full, res
    return full
